# revision 1
# baseline (speedup 1.0000x reference)
"""Trainium2 Bass kernel for the NCT system-simulator rollout.

Math: each RK4 step with state-feedback control held fixed over stages is an
affine map s' = A s + d per environment, with A = R + b*(S e2) w^T and d =
(b*bias + sin b)*(S e2), where R, S are Taylor matrices of the constant
dynamics Jacobian M = [[0,1],[1,0]].  Over num_steps identical affine steps,
the accumulated quadratic reward is evaluated in closed form via the
eigenvalues of A (trace/det are affine in b; eigenvalues real and
well-separated for these dynamics) and five geometric sums computed with
log2(N) product-doubling:  sum_{n<N} x^n = (1+x) * prod_{k>=1} (1 + x^(2^k)).

Data parallel over 8 NeuronCores: 8192 envs per core as [128 part x 64 free].
Engine split: DVE does the elementwise algebra; ScalarE (ACT) runs sin,
ln/exp-based reciprocals, squares and the power-doubling chains concurrently.
"""

import math

import numpy as np

B_TOTAL = 65536
NCORES = 8
BCORE = B_TOTAL // NCORES  # 8192
P = 128
FD = BCORE // P  # 64
NSTEPS = 512
NDOUBLE = 9  # log2(NSTEPS)
DT = 0.01
P1, P2, PA = 1.0, 0.1, 0.01
TERM = 10.0

CST_W = 32

# const column layout: two 9-column affine patterns (AFF = C1A*b + C0A), then
# scalar columns.  AFF blocks: a11 a12 a21 a22 t/2 uq detIn wv1n wv2n
C_C1A = 0  # 0..8
C_C0A = 9  # 9..17
C_W3 = 18  # 18..20: w1 w2 w3
C_TW = 21  # 21..22: -TERM*P1, -TERM*P2
C_Q0 = 23
C_BIAS = 24
C_W0 = 25
C_W1 = 26


def _host_consts(W, bias):
    """O(1) scalar preprocessing of the replicated controller params."""
    h = DT
    c = 1 + h**2 / 2 + h**4 / 24
    s = h + h**3 / 6
    Se1 = h * (h / 2 + h**3 / 24)
    Se2 = h * (1 + h**2 / 6)
    W0, W1 = float(W[0, 0]), float(W[1, 0])
    bias = float(bias)
    ka11, ka12, ka21, ka22 = Se1 * W0, Se1 * W1, Se2 * W0, Se2 * W1
    kt2 = (ka11 + ka22) / 2  # t/2 = c + kt2*b
    q0 = 4 * s * s
    q1 = 4 * s * (ka12 + ka21)
    q2 = (ka11 - ka22) ** 2 + 4 * ka12 * ka21  # disc = q0 + q1 b + q2 b^2
    kd = c * (ka11 + ka22) - s * (ka12 + ka21)  # detA = (c^2-s^2) + kd*b
    e0 = (c * c - s * s - 1) - (2 * c - 2)  # det(I-A) = e0 + e1*b (stable)
    e1 = kd - (ka11 + ka22)
    # adj(I-A) @ Se2vec is affine in b: (al_i + be_i*b).  detIn = -det(I-A)
    # is positive for these dynamics; the sign is folded into wv.
    al1 = (1 - c) * Se1 + s * Se2
    be1 = -ka22 * Se1 + ka12 * Se2
    al2 = s * Se1 + (1 - c) * Se2
    be2 = ka21 * Se1 - ka11 * Se2
    w1, w2, w3 = -DT * P1, -DT * P2, -DT * PA

    cols = np.zeros(CST_W, dtype=np.float64)
    cols[C_C1A : C_C1A + 9] = [ka11, ka12, ka21, ka22, kt2, q2, -e1, -be1, -be2]
    cols[C_C0A : C_C0A + 9] = [c, s, s, c, c, q1, -e0, -al1, -al2]
    cols[C_W3 : C_W3 + 3] = [w1, w2, w3]
    cols[C_TW : C_TW + 2] = [-TERM * P1, -TERM * P2]
    cols[C_Q0] = q0
    cols[C_BIAS] = bias
    cols[C_W0] = W0
    cols[C_W1] = W1
    return np.tile(cols.astype(np.float32), (P, 1))  # [128, CST_W]


def _hoist_extra_waits(nc, keep_attached=1):
    """This toolchain's codegen allows at most one attached sync-wait per
    instruction; move extra waits onto standalone EventSemaphore instructions
    (the encoding raw-Bass wait_ge uses) inserted just before the consumer."""
    import concourse.mybir as mybir

    wid = [0]
    for fn in nc.m.functions:
        for bb in fn.blocks:
            insts = list(bb.instructions)
            if not any(
                i.sync_info and i.sync_info.on_wait and len(i.sync_info.on_wait) > keep_attached
                for i in insts
            ):
                continue
            new = []
            for inst in insts:
                si = inst.sync_info
                waits = list(si.on_wait) if si and si.on_wait else []
                if len(waits) > keep_attached:
                    hoist, keep = waits[: len(waits) - keep_attached], waits[len(waits) - keep_attached :]
                    for w in hoist:
                        ev = mybir.InstEventSemaphore(
                            name=f"HW-{wid[0]}", ins=[], outs=[]
                        )
                        wid[0] += 1
                        ev.engine = inst.engine
                        ev.sync_info = mybir.SyncInfo(on_wait=[w], on_update=[])
                        try:
                            nc.register_instruction(ev, overwrite=True)
                        except Exception:
                            pass
                        new.append(ev)
                    si.on_wait = keep
                new.append(inst)
            bb.instructions = new


def build_nc(debug_outputs=False):
    import concourse.bass as bass
    import concourse.mybir as mybir
    from concourse.tile import TileContext

    Alu = mybir.AluOpType
    Act = mybir.ActivationFunctionType
    f32 = mybir.dt.float32

    nc = bass.Bass(
        "TRN2", target_bir_lowering=False, debug=False, num_devices=NCORES
    )
    s0d = nc.dram_tensor("s0", [BCORE, 2], f32, kind="ExternalInput")
    bpd = nc.dram_tensor("bp", [BCORE], f32, kind="ExternalInput")
    cstd = nc.dram_tensor("cst", [P, CST_W], f32, kind="ExternalInput")
    outd = nc.dram_tensor("out", [BCORE], f32, kind="ExternalOutput")
    dbg_tensors = {}

    V = nc.vector
    S = nc.scalar

    with TileContext(nc) as tc:
        with tc.tile_pool(name="main", bufs=1) as pool:
            cst = pool.tile([P, CST_W], f32)
            bS = pool.tile([P, FD], f32)
            s0S = pool.tile([P, 2 * FD], f32)
            nc.sync.dma_start(cst[:], cstd.ap())
            nc.sync.dma_start(bS[:], bpd.ap().rearrange("(p f) -> p f", p=P))
            nc.sync.dma_start(
                s0S[:], s0d.ap().rearrange("(p f) t -> p (f t)", p=P)
            )

            def col(i):
                return cst[:, i : i + 1]

            def b2(ap):
                return ap.unsqueeze(1).broadcast_to([P, 2, FD])

            def b3(ap):
                return ap.unsqueeze(1).broadcast_to([P, 3, FD])

            def blk(t, i, n=1):
                return t[:, i * FD : (i + n) * FD]

            def v2(ap):
                return ap.rearrange("p (k f) -> p k f", k=2)

            def v3(ap):
                return ap.rearrange("p (k f) -> p k f", k=3)

            # ---- all affine-in-b quantities in one pair of fused ops ----
            AFF = pool.tile([P, 9 * FD], f32)
            A9 = AFF[:].rearrange("p (k f) -> p k f", k=9)
            c1pat = cst[:, C_C1A : C_C1A + 9].unsqueeze(2).broadcast_to([P, 9, FD])
            c0pat = cst[:, C_C0A : C_C0A + 9].unsqueeze(2).broadcast_to([P, 9, FD])
            b9 = bS[:].unsqueeze(1).broadcast_to([P, 9, FD])
            V.tensor_mul(A9, c1pat, b9)
            V.tensor_add(A9, A9, c0pat)
            A4 = AFF[:, 0 : 4 * FD].rearrange("p (k f) -> p k f", k=4)
            t2 = blk(AFF, 4)
            uq = blk(AFF, 5)
            detIn = blk(AFF, 6)

            # ---- 1/sqrt(disc) and 1/detIn on ACT via ln/exp ----
            mq = pool.tile([P, FD], f32)
            V.tensor_mul(mq[:], uq, bS[:])
            # rs0 = 1/sqrt(disc), rdp0 = 1/detIn via ln/exp on ACT, packed in
            # one tile so both Newton corrections finish in one FD128 multiply.
            lnd = pool.tile([P, FD], f32)
            S.activation(lnd[:], mq[:], Act.Ln, bias=col(C_Q0))
            lnD = pool.tile([P, FD], f32)
            S.activation(lnD[:], detIn, Act.Ln)
            rr = pool.tile([P, 2 * FD], f32)
            S.activation(blk(rr, 0), lnd[:], Act.Exp, scale=-0.5)
            S.activation(blk(rr, 1), lnD[:], Act.Exp, scale=-1.0)
            # Newton: rs = rs0*(1.5 - 0.5*disc*rs0^2); rdp = rdp0*(2 - detIn*rdp0).
            # The HW exp/ln LUT error (~2e-5) would otherwise be amplified
            # 1024x into the geometric sums via the eigenvalues.
            rp = pool.tile([P, FD], f32)
            V.tensor_mul(rp[:], blk(rr, 0), blk(rr, 0))
            rdp2 = pool.tile([P, FD], f32)
            V.scalar_tensor_tensor(rdp2[:], mq[:], col(C_Q0), rp[:], Alu.add, Alu.mult)
            ff = pool.tile([P, 2 * FD], f32)
            V.tensor_scalar(blk(ff, 0), rdp2[:], -0.5, 1.5, Alu.mult, Alu.add)
            md = pool.tile([P, FD], f32)
            V.tensor_mul(md[:], detIn, blk(rr, 1))
            V.tensor_scalar(blk(ff, 1), md[:], -1.0, 2.0, Alu.mult, Alu.add)
            rsrd = pool.tile([P, 2 * FD], f32)
            V.tensor_mul(rsrd[:], rr[:], ff[:])
            rs = blk(rsrd, 0)
            rdp = blk(rsrd, 1)

            # ---- sin(b) via double angle: sin b = sin(b/2) * (2 - 4 sin^2(b/4))
            sa = pool.tile([P, FD], f32)
            S.activation(sa[:], bS[:], Act.Sin, scale=0.5)
            sb = pool.tile([P, FD], f32)
            S.activation(sb[:], bS[:], Act.Sin, scale=0.25)
            sb2 = pool.tile([P, FD], f32)
            S.activation(sb2[:], sb[:], Act.Square)


            # ---- eigenvalues ----
            sq = pool.tile([P, FD], f32)  # lam1 - lam2 = sqrt(disc)
            V.scalar_tensor_tensor(sq[:], mq[:], col(C_Q0), rs, Alu.add, Alu.mult)
            lam = pool.tile([P, 2 * FD], f32)
            V.scalar_tensor_tensor(blk(lam, 0), sq[:], 0.5, t2, Alu.mult, Alu.add)
            V.scalar_tensor_tensor(blk(lam, 1), sq[:], -0.5, t2, Alu.mult, Alu.add)

            # ---- power-doubling chains on ACT: levels k = (l1^2, l2^2, l1 l2)^(2^k)
            xp = pool.tile([P, 9 * 3 * FD], f32)

            def lvl(k, i=0, n=3):
                return xp[:, (3 * k + i) * FD : (3 * k + i + n) * FD]

            V.tensor_mul(lvl(0, 2, 1), blk(lam, 0), blk(lam, 1))
            S.activation(lvl(0, 0, 2), lam[:], Act.Square)
            for k in range(1, 9):
                S.activation(lvl(k), lvl(k - 1), Act.Square)
            # level 8 = (l1^512, l2^512, (l1 l2)^256)

            # ---- fixed point s* ----
            cbv = pool.tile([P, FD], f32)
            S.activation(cbv[:], sb2[:], Act.Copy, bias=2.0, scale=-4.0)
            sinb = pool.tile([P, FD], f32)
            V.tensor_mul(sinb[:], sa[:], cbv[:])
            z = pool.tile([P, FD], f32)
            V.scalar_tensor_tensor(z[:], bS[:], col(C_BIAS), sinb[:], Alu.mult, Alu.add)
            zr = pool.tile([P, FD], f32)
            V.tensor_mul(zr[:], z[:], rdp)
            S9 = pool.tile([P, 9 * FD], f32)  # (kap1..3 | rho1..3 | sig1..3)
            V.tensor_mul(v2(blk(S9, 0, 2)), b2(zr[:]), v2(blk(AFF, 7, 2)))

            # ---- Krylov vectors: UV = (AAy1 AAy2 u3 Ay1 Ay2 v3 y1 y2) ----
            UV = pool.tile([P, 8 * FD], f32)
            V.tensor_sub(
                v2(blk(UV, 6, 2)),
                s0S[:].rearrange("p (f t) -> p t f", t=2),
                v2(blk(S9, 0, 2)),
            )
            yrep = (
                blk(UV, 6, 2)
                .rearrange("p (t f) -> p t f", t=2)
                .unsqueeze(1)
                .broadcast_to([P, 2, 2, FD])
            )
            Pp = pool.tile([P, 4 * FD], f32)
            Pp4 = Pp[:].rearrange("p (i t f) -> p i t f", i=2, t=2)
            V.tensor_mul(Pp4, A4.rearrange("p (i t) f -> p i t f", i=2), yrep)
            V.tensor_add(v2(blk(UV, 3, 2)), Pp4[:, :, 0, :], Pp4[:, :, 1, :])
            Ayrep = (
                blk(UV, 3, 2)
                .rearrange("p (t f) -> p t f", t=2)
                .unsqueeze(1)
                .broadcast_to([P, 2, 2, FD])
            )
            Pp2 = pool.tile([P, 4 * FD], f32)
            Pp24 = Pp2[:].rearrange("p (i t f) -> p i t f", i=2, t=2)
            V.tensor_mul(Pp24, A4.rearrange("p (i t) f -> p i t f", i=2), Ayrep)
            V.tensor_add(v2(blk(UV, 0, 2)), Pp24[:, :, 0, :], Pp24[:, :, 1, :])

            # u3 = W0*Ay1 + W1*Ay2, v3 = W0*y1 + W1*y2 in one fused pair
            t3ab = pool.tile([P, 2 * FD], f32)
            uv8 = UV[:].rearrange("p (k f) -> p k f", k=8)
            S.activation(
                v2(t3ab[:]), uv8[:, 3::3, :], Act.Copy, scale=col(C_W0)
            )
            # out blocks (2,5), in0 blocks (4,7): both stride 3*FD apart
            V.scalar_tensor_tensor(
                uv8[:, 2::3, :],
                uv8[:, 4::3, :],
                col(C_W1),
                v2(t3ab[:]),
                Alu.mult,
                Alu.add,
            )
            # kap3 = W0*s*1 + W1*s*2 + bias
            t3c = pool.tile([P, FD], f32)
            S.activation(
                t3c[:], blk(S9, 0), Act.Identity, bias=col(C_BIAS), scale=col(C_W0)
            )
            V.scalar_tensor_tensor(
                blk(S9, 2), blk(S9, 1), col(C_W1), t3c[:], Alu.mult, Alu.add
            )

            # ---- rho = (u - lam2 v) rs ; sig = v - rho ----
            mrho = pool.tile([P, 3 * FD], f32)
            V.tensor_mul(v3(mrho[:]), b3(blk(lam, 1)), v3(blk(UV, 3, 3)))
            rhon = pool.tile([P, 3 * FD], f32)
            V.tensor_sub(rhon[:], blk(UV, 0, 3), mrho[:])
            V.tensor_mul(v3(blk(S9, 3, 3)), v3(rhon[:]), b3(rs))
            V.tensor_sub(blk(S9, 6, 3), blk(UV, 3, 3), blk(S9, 3, 3))

            # ---- geometric sums: P = prod_{k=0..7} (1 + level_k) ----
            g = pool.tile([P, 3 * FD], f32)
            V.tensor_scalar(g[:], lvl(0), 1.0, None, Alu.add)
            for k in range(1, 8):
                V.scalar_tensor_tensor(g[:], lvl(k), 1.0, g[:], Alu.add, Alu.mult)
            # Gamma buf: (N, G11, G22, G1, G2, G12); x2 on cross terms is folded
            # into the C1/C2 pair products below.
            Gbuf = pool.tile([P, 6 * FD], f32)
            V.memset(blk(Gbuf, 0), float(NSTEPS))
            V.scalar_tensor_tensor(
                blk(Gbuf, 1, 2), lvl(8, 0, 2), 1.0, g[:, 0 : 2 * FD], Alu.add, Alu.mult
            )
            V.scalar_tensor_tensor(
                blk(Gbuf, 3, 2), lam[:], 1.0, g[:, 0 : 2 * FD], Alu.add, Alu.mult
            )
            V.scalar_tensor_tensor(
                blk(Gbuf, 5), lvl(8, 2, 1), 1.0, g[:, 2 * FD :], Alu.add, Alu.mult
            )

            # ---- pair products and contraction ----
            Sh9 = pool.tile([P, 9 * FD], f32)
            w3pat = (
                cst[:, C_W3 : C_W3 + 3]
                .unsqueeze(1)
                .unsqueeze(3)
                .broadcast_to([P, 3, 3, FD])
            )
            S9g = S9[:].rearrange("p (g j f) -> p g j f", g=3, j=3)
            V.tensor_mul(Sh9[:].rearrange("p (g j f) -> p g j f", g=3, j=3), S9g, w3pat)
            Kbuf = pool.tile([P, 18 * FD], f32)
            V.tensor_mul(blk(Kbuf, 0, 9), Sh9[:], S9[:])
            V.scalar_tensor_tensor(
                blk(Kbuf, 9, 6).rearrange("p (k f) -> p k f", k=2),
                blk(Sh9, 0, 3).unsqueeze(1).broadcast_to([P, 2, 3 * FD]),
                2.0,
                blk(S9, 3, 6).rearrange("p (k f) -> p k f", k=2),
                Alu.mult,
                Alu.mult,
            )
            V.scalar_tensor_tensor(
                blk(Kbuf, 15, 3), blk(Sh9, 3, 3), 2.0, blk(S9, 6, 3), Alu.mult, Alu.mult
            )
            Xbuf = pool.tile([P, 18 * FD], f32)
            grep = (
                Gbuf[:]
                .rearrange("p (k f) -> p k f", k=6)
                .unsqueeze(2)
                .broadcast_to([P, 6, 3, FD])
            )
            V.tensor_mul(
                Xbuf[:].rearrange("p (f k j) -> p k j f", k=6, j=3),
                Kbuf[:].rearrange("p (k j f) -> p k j f", k=6, j=3),
                grep,
            )
            tot = pool.tile([P, FD], f32)
            V.tensor_reduce(
                tot[:],
                Xbuf[:].rearrange("p (f k) -> p f k", k=18),
                axis=mybir.AxisListType.X,
                op=Alu.add,
            )

            # ---- terminal state penalty ----
            d12 = pool.tile([P, FD], f32)
            V.tensor_sub(d12[:], lvl(8, 0, 1), lvl(8, 1, 1))
            pq = pool.tile([P, 2 * FD], f32)
            V.tensor_mul(blk(pq, 0), d12[:], rs)
            mq2 = pool.tile([P, FD], f32)
            V.tensor_mul(mq2[:], blk(pq, 0), blk(lam, 1))
            V.tensor_sub(blk(pq, 1), lvl(8, 1, 1), mq2[:])
            m1 = pool.tile([P, 2 * FD], f32)
            V.tensor_mul(v2(m1[:]), b2(blk(pq, 0)), v2(blk(UV, 3, 2)))
            m2 = pool.tile([P, 2 * FD], f32)
            V.tensor_mul(v2(m2[:]), b2(blk(pq, 1)), v2(blk(UV, 6, 2)))
            sN = pool.tile([P, 2 * FD], f32)
            V.tensor_add(sN[:], blk(S9, 0, 2), m1[:])
            V.tensor_add(sN[:], sN[:], m2[:])
            sqN = pool.tile([P, 2 * FD], f32)
            S.activation(sqN[:], sN[:], Act.Square)
            tw = pool.tile([P, 2 * FD], f32)
            V.tensor_mul(
                v2(tw[:]),
                v2(sqN[:]),
                cst[:, C_TW : C_TW + 2].unsqueeze(2).broadcast_to([P, 2, FD]),
            )
            tots = pool.tile([P, FD], f32)
            V.tensor_add(tots[:], blk(tw, 0), blk(tw, 1))
            outT = pool.tile([P, FD], f32)
            V.tensor_add(outT[:], tot[:], tots[:])
            nc.sync.dma_start(outd.ap().rearrange("(p f) -> p f", p=P), outT[:])

            if debug_outputs:
                for nm, t in [
                    ("dAFF", AFF), ("dlam", lam), ("drsrd", rsrd),
                    ("dsinb", sinb), ("dS9", S9), ("dUV", UV), ("dg", g),
                    ("dG", Gbuf), ("dK", Kbuf), ("dtot", tot), ("dsN", sN),
                ]:
                    w = t.shape[1] if hasattr(t, "shape") else None
                    d = nc.dram_tensor(nm, [P, w], f32, kind="ExternalOutput")
                    dbg_tensors[nm] = d
                    nc.sync.dma_start(d.ap(), t[:])

    _hoist_extra_waits(nc)
    return nc


_NC_CACHE = None
TRACE = False
LAST_RESULT = None


def _get_nc():
    global _NC_CACHE
    if _NC_CACHE is None:
        _NC_CACHE = build_nc()
    return _NC_CACHE


def kernel(initial_states, b_param, W, bias, num_steps):
    from concourse.bass_utils import run_bass_kernel_spmd

    assert int(num_steps) == NSTEPS, f"kernel compiled for num_steps={NSTEPS}"
    s0 = np.ascontiguousarray(np.asarray(initial_states, dtype=np.float32))
    bp = np.ascontiguousarray(np.asarray(b_param, dtype=np.float32)).reshape(-1)
    assert s0.shape == (B_TOTAL, 2) and bp.shape == (B_TOTAL,)
    cst = _host_consts(
        np.asarray(W, dtype=np.float64),
        np.asarray(bias, dtype=np.float64).reshape(-1)[0],
    )

    in_maps = []
    for c in range(NCORES):
        lo, hi = c * BCORE, (c + 1) * BCORE
        in_maps.append(
            {
                "s0": np.ascontiguousarray(s0[lo:hi]),
                "bp": np.ascontiguousarray(bp[lo:hi]),
                "cst": cst,
            }
        )

    nc = _get_nc()
    res = run_bass_kernel_spmd(
        nc, in_maps, core_ids=list(range(NCORES)), trace=TRACE
    )
    global LAST_RESULT
    LAST_RESULT = res
    out = np.concatenate([res.results[c]["out"] for c in range(NCORES)])
    return out.reshape(B_TOTAL, 1).astype(np.float32)

